# revision 2
# baseline (speedup 1.0000x reference)
"""Biaffine kernel for Trainium2, 8-core SPMD — v2 (bf16, fold-free).

Math (reference):
    out[b,x,y,o] = bwn0 * sum_{i,j<=512} x1b[b,x,i] W_bil[o,i,j] x2b[b,y,j]
                 + bwn1 * (x1@W_lin[:512] [b,x,o] + x2@W_lin[512:] [b,y,o] + b_lin[o])
    with x1b/x2b = x append-ones, bwn = softmax(bw).

Decomposition (exact):
    UT'[b,o][j,x] = sum_i (bwn0*W_bil[o,i,j]) * x1[b,x,i] + V[j,o]
    out[b,o][x,y] = sum_j UT'[b,o][j,x] * x2[b,y,j] + D1X[b][x,o]
      - the V term expands to sum_j V[j,o]*x2[b,y,j] = D2[b,y,o]  (free!)
      - D1X[b][x,o] = sum_i x1[b,x,i]*G[i,o] + g0[o] is a per-partition
        scalar in the [x-part, y-free] output layout -> rides the eviction.
    G = bwn0*W_bil[o,:512,512] + bwn1*W_lin[:512,o]
    V = bwn0*W_bil[o,512,:512] + bwn1*W_lin[512:,o]
    g0 = bwn0*W_bil[o,512,512] + bwn1*b_lin[o]

Sharding: tensor-parallel over O (128 output channels -> 16 per core).
All matmuls in bf16 (1 cycle/row on PE, same as fp32r, but half the
DMA/SBUF); PSUM accumulates fp32; output stored bf16, upcast on host.
Program order interleaves A(og+1) with B(og) so the PE never waits on
an eviction, keeping the tensor engine at its boosted clock.
"""

import numpy as np
import ml_dtypes

import concourse.bass as bass
import concourse.mybir as mybir
import concourse.tile as tile
from concourse.bass_utils import run_bass_kernel_spmd

B, L, D, O = 4, 256, 512, 128
N_CORES = 8
O_LOC = O // N_CORES          # 16 output channels per core
F32 = mybir.dt.float32
BF16 = mybir.dt.bfloat16
BL = B * L


# --------------------------------------------------------------------------
# Workaround: this container's walrus build accepts only ONE sync wait per
# instruction ("Too many sync wait commands").  Tile's wait assignment can
# attach several.  Post-pass: hoist extra waits onto InstEventSemaphore
# wait-carriers inserted immediately before the instruction on the same
# engine stream (same stall point, identical semantics).
_WS_CTR = [0]


def _split_multi_waits(nc):
    for f in nc.m.functions:
        for blk in f.blocks:
            insts = blk.instructions
            new = []
            changed = False
            for inst in insts:
                si = inst.sync_info
                waits = list(si.on_wait) if (si and si.on_wait) else []
                if len(waits) > 1:
                    for w in waits[:-1]:
                        _WS_CTR[0] += 1
                        carrier = mybir.InstEventSemaphore(
                            name=f"waitsplit_{_WS_CTR[0]}", ins=[], outs=[]
                        )
                        carrier.engine = inst.engine
                        carrier.sync_info = mybir.SyncInfo(on_wait=[w], on_update=[])
                        new.append(carrier)
                    si.on_wait = [waits[-1]]
                    changed = True
                new.append(inst)
            if changed:
                blk.instructions = new


# --------------------------------------------------------------------------
def build_nc(split_waits=True):
    nc = bass.Bass("TRN2", target_bir_lowering=False, debug=False,
                   num_devices=N_CORES)

    WMD = nc.dram_tensor("WMD", [O_LOC, D, D], BF16, kind="ExternalInput").ap()
    X1T = nc.dram_tensor("X1T", [D, BL], BF16, kind="ExternalInput").ap()
    X2T = nc.dram_tensor("X2T", [D, BL], BF16, kind="ExternalInput").ap()
    GD = nc.dram_tensor("GD", [D, O_LOC], BF16, kind="ExternalInput").ap()
    VD = nc.dram_tensor("VD", [D, O_LOC], F32, kind="ExternalInput").ap()
    G0D = nc.dram_tensor("G0D", [1, O_LOC], BF16, kind="ExternalInput").ap()
    ONE1 = nc.dram_tensor("ONE1", [1, 128], BF16, kind="ExternalInput").ap()
    OUT = nc.dram_tensor("OUT", [B, O_LOC, L, L], BF16, kind="ExternalOutput").ap()

    with tile.TileContext(nc) as tc:
        with (
            tc.tile_pool(name="const", bufs=1) as cst,
            tc.tile_pool(name="w", bufs=3) as wpool,
            tc.tile_pool(name="ut", bufs=3) as utpool,
            tc.tile_pool(name="cs", bufs=8) as cspool,
        ):
            # ---- resident inputs (order = DMA queue order; X1/G/W0 first so
            # the D1X precompute and A(0) can start as early as possible) -----
            Ws_tiles = {}
            UT_tiles = {}

            def dma_w(og):
                ws = wpool.tile([128, 4, D], BF16, tag="ws", name=f"ws{og}")
                nc.sync.dma_start(
                    out=ws[:],
                    in_=WMD[og].rearrange("(it p) j -> p it j", p=128),
                )
                Ws_tiles[og] = ws

            X1s = cst.tile([128, 4, BL], BF16, tag="x1s")     # [i%128, it, b*256+x]
            nc.sync.dma_start(out=X1s[:], in_=X1T.rearrange("(it p) c -> p it c", p=128))
            Gs = cst.tile([128, 4, O_LOC], BF16, tag="gs")
            nc.sync.dma_start(out=Gs[:], in_=GD.rearrange("(it p) o -> p it o", p=128))
            g0row = cst.tile([1, O_LOC], BF16, tag="g0row")
            nc.sync.dma_start(out=g0row[:], in_=G0D[:])
            one1 = cst.tile([1, 128], BF16, tag="one1")
            nc.sync.dma_start(out=one1[:], in_=ONE1[:])
            dma_w(0)
            Vs = cst.tile([128, 4, O_LOC], F32, tag="vs")
            nc.sync.dma_start(out=Vs[:], in_=VD.rearrange("(jt p) o -> p jt o", p=128))
            dma_w(1)
            X2s = cst.tile([128, 4, BL], BF16, tag="x2s")     # [j%128, jt, b*256+y]
            nc.sync.dma_start(out=X2s[:], in_=X2T.rearrange("(jt p) c -> p jt c", p=128))

            # D1X[b][x-part, xt, o] incl. +g0 (via rank-1 K=1 matmul)
            D1X = cst.tile([128, B, 2, O_LOC], F32, tag="d1x")
            with tc.tile_pool(name="psD", bufs=2, space="PSUM") as psD:
                for b in range(B):
                    for xt in range(2):
                        pd = psD.tile([128, O_LOC], F32, tag="pd")
                        c0 = b * L + xt * 128
                        for it in range(4):
                            nc.tensor.matmul(
                                pd[:],
                                lhsT=X1s[:, it, c0:c0 + 128],
                                rhs=Gs[:, it, :],
                                start=(it == 0), stop=False,
                            )
                        nc.tensor.matmul(
                            pd[:],
                            lhsT=one1[0:1, :],
                            rhs=g0row[0:1, :],
                            start=False, stop=True,
                        )
                        nc.scalar.copy(out=D1X[:, b, xt, :], in_=pd[:])

            # ---- main loop: A(og) interleaved with B(og-1) ------------------
            psA_ctx = tc.tile_pool(name="psA", bufs=4, space="PSUM")
            psB_ctx = tc.tile_pool(name="psB", bufs=4, space="PSUM")
            psA = psA_ctx.__enter__()
            psB = psB_ctx.__enter__()

            def step_a(og):
                # UT'[j-part, jt, c] = sum_i W[i,j] x1[i,c]  (+V on evict)
                ut = utpool.tile([128, 4, BL], BF16, tag="ut", name=f"ut{og}")
                ws = Ws_tiles.pop(og)
                ev = 0
                for jt in range(4):
                    for bp in range(2):
                        pa = psA.tile([128, 512], F32, tag="pa")
                        for it in range(4):
                            nc.tensor.matmul(
                                pa[:],
                                lhsT=ws[:, it, jt * 128:(jt + 1) * 128],
                                rhs=X1s[:, it, bp * 512:(bp + 1) * 512],
                                start=(it == 0), stop=(it == 3),
                            )
                        # evict + V[j, og]; alternate DVE/Act to balance
                        dst = ut[:, jt, bp * 512:(bp + 1) * 512]
                        if ev % 2 == 0:
                            nc.vector.tensor_scalar_add(
                                dst, pa[:], Vs[:, jt, og:og + 1])
                        else:
                            nc.scalar.add(dst, pa[:], Vs[:, jt, og:og + 1])
                        ev += 1
                UT_tiles[og] = ut

            def step_b(og):
                ut = UT_tiles.pop(og)
                ev = 0
                for b in range(B):
                    pb = psB.tile([128, 512], F32, tag="pb")
                    for xt in range(2):
                        for jt in range(4):
                            nc.tensor.matmul(
                                pb[:, xt * 256:(xt + 1) * 256],
                                lhsT=ut[:, jt, b * L + xt * 128: b * L + xt * 128 + 128],
                                rhs=X2s[:, jt, b * L:(b + 1) * L],
                                start=(jt == 0), stop=(jt == 3),
                            )
                    for xt in range(2):
                        cs = cspool.tile([128, 256], BF16, tag="cs")
                        src = pb[:, xt * 256:(xt + 1) * 256]
                        bias = D1X[:, b, xt, og:og + 1]
                        if ev % 2 == 0:
                            nc.scalar.add(cs[:], src, bias)
                        else:
                            nc.vector.tensor_scalar_add(cs[:], src, bias)
                        ev += 1
                        nc.sync.dma_start(
                            out=OUT[b, og, xt * 128:(xt + 1) * 128, :],
                            in_=cs[:],
                        )

            step_a(0)
            for og in range(1, O_LOC):
                if og + 1 < O_LOC:
                    dma_w(og + 1)
                step_a(og)
                step_b(og - 1)
            step_b(O_LOC - 1)

            psB_ctx.__exit__(None, None, None)
            psA_ctx.__exit__(None, None, None)

    if split_waits:
        _split_multi_waits(nc)
    return nc


_NC_CACHE = None


def _get_nc():
    global _NC_CACHE
    if _NC_CACHE is None:
        _NC_CACHE = build_nc()
    return _NC_CACHE


def _prep_inputs(x1, x2, bw, W_bil, W_lin, b_lin):
    """Host-side glue: softmax of the 2-vector, per-core slicing/layout."""
    x1 = np.asarray(x1, np.float32)
    x2 = np.asarray(x2, np.float32)
    bw = np.asarray(bw, np.float64)
    W_bil = np.asarray(W_bil, np.float32)
    W_lin = np.asarray(W_lin, np.float32)
    b_lin = np.asarray(b_lin, np.float32)

    e = np.exp(bw - bw.max())
    bwn = (e / e.sum()).astype(np.float32)
    bwn0, bwn1 = float(bwn[0]), float(bwn[1])

    bf = ml_dtypes.bfloat16
    x1T = np.ascontiguousarray(x1.transpose(2, 0, 1).reshape(D, BL)).astype(bf)
    x2T = np.ascontiguousarray(x2.transpose(2, 0, 1).reshape(D, BL)).astype(bf)
    one1 = np.ones((1, 128), bf)

    in_maps = []
    for c in range(N_CORES):
        o_sl = slice(c * O_LOC, (c + 1) * O_LOC)
        Wb = W_bil[o_sl]                                   # [16, 513, 513]
        WMD = np.ascontiguousarray(bwn0 * Wb[:, :D, :D]).astype(bf)
        G = (bwn0 * Wb[:, :D, D].T + bwn1 * W_lin[:D, o_sl]).astype(bf)
        V = (bwn0 * Wb[:, D, :D].T + bwn1 * W_lin[D:, o_sl]).astype(np.float32)
        G0 = (bwn0 * Wb[:, D, D] + bwn1 * b_lin[o_sl]).reshape(1, O_LOC).astype(bf)
        in_maps.append({
            "WMD": WMD, "X1T": x1T, "X2T": x2T,
            "GD": np.ascontiguousarray(G),
            "VD": np.ascontiguousarray(V),
            "G0D": np.ascontiguousarray(G0), "ONE1": one1,
        })
    return in_maps


def _assemble(results):
    out = np.empty((B, L, L, O), np.float32)
    for c in range(N_CORES):
        # per-core OUT is [b, o_local, x, y] -> full is [b, x, y, o]
        out[:, :, :, c * O_LOC:(c + 1) * O_LOC] = \
            np.asarray(results[c]["OUT"]).astype(np.float32).transpose(0, 2, 3, 1)
    return out


def kernel(**inputs):
    in_maps = _prep_inputs(**inputs)
    nc = _get_nc()
    res = run_bass_kernel_spmd(nc, in_maps, list(range(N_CORES)))
    return _assemble(res.results)


# revision 3
# speedup vs baseline: 1.0182x; 1.0182x over previous
"""Biaffine kernel for Trainium2, 8-core SPMD — v2 (bf16, fold-free).

Math (reference):
    out[b,x,y,o] = bwn0 * sum_{i,j<=512} x1b[b,x,i] W_bil[o,i,j] x2b[b,y,j]
                 + bwn1 * (x1@W_lin[:512] [b,x,o] + x2@W_lin[512:] [b,y,o] + b_lin[o])
    with x1b/x2b = x append-ones, bwn = softmax(bw).

Decomposition (exact):
    UT'[b,o][j,x] = sum_i (bwn0*W_bil[o,i,j]) * x1[b,x,i] + V[j,o]
    out[b,o][x,y] = sum_j UT'[b,o][j,x] * x2[b,y,j] + D1X[b][x,o]
      - the V term expands to sum_j V[j,o]*x2[b,y,j] = D2[b,y,o]  (free!)
      - D1X[b][x,o] = sum_i x1[b,x,i]*G[i,o] + g0[o] is a per-partition
        scalar in the [x-part, y-free] output layout -> rides the eviction.
    G = bwn0*W_bil[o,:512,512] + bwn1*W_lin[:512,o]
    V = bwn0*W_bil[o,512,:512] + bwn1*W_lin[512:,o]
    g0 = bwn0*W_bil[o,512,512] + bwn1*b_lin[o]

Sharding: tensor-parallel over O (128 output channels -> 16 per core).
All matmuls in bf16 (1 cycle/row on PE, same as fp32r, but half the
DMA/SBUF); PSUM accumulates fp32; output stored bf16, upcast on host.
Program order interleaves A(og+1) with B(og) so the PE never waits on
an eviction, keeping the tensor engine at its boosted clock.
"""

import numpy as np
import ml_dtypes

import concourse.bass as bass
import concourse.mybir as mybir
import concourse.tile as tile
from concourse.bass_utils import run_bass_kernel_spmd

B, L, D, O = 4, 256, 512, 128
N_CORES = 8
O_LOC = O // N_CORES          # 16 output channels per core
F32 = mybir.dt.float32
BF16 = mybir.dt.bfloat16
BL = B * L


# --------------------------------------------------------------------------
# Workaround: this container's walrus build accepts only ONE sync wait per
# instruction ("Too many sync wait commands").  Tile's wait assignment can
# attach several.  Post-pass: hoist extra waits onto InstEventSemaphore
# wait-carriers inserted immediately before the instruction on the same
# engine stream (same stall point, identical semantics).
_WS_CTR = [0]


def _split_multi_waits(nc):
    for f in nc.m.functions:
        for blk in f.blocks:
            insts = blk.instructions
            new = []
            changed = False
            for inst in insts:
                si = inst.sync_info
                waits = list(si.on_wait) if (si and si.on_wait) else []
                if len(waits) > 1:
                    for w in waits[:-1]:
                        _WS_CTR[0] += 1
                        carrier = mybir.InstEventSemaphore(
                            name=f"waitsplit_{_WS_CTR[0]}", ins=[], outs=[]
                        )
                        carrier.engine = inst.engine
                        carrier.sync_info = mybir.SyncInfo(on_wait=[w], on_update=[])
                        new.append(carrier)
                    si.on_wait = [waits[-1]]
                    changed = True
                new.append(inst)
            if changed:
                blk.instructions = new


# --------------------------------------------------------------------------
def build_nc(split_waits=True):
    nc = bass.Bass("TRN2", target_bir_lowering=False, debug=False,
                   num_devices=N_CORES)

    WMD = nc.dram_tensor("WMD", [O_LOC, D, D], BF16, kind="ExternalInput").ap()
    X1T = nc.dram_tensor("X1T", [D, BL], BF16, kind="ExternalInput").ap()
    X2T = nc.dram_tensor("X2T", [D, BL], BF16, kind="ExternalInput").ap()
    GD = nc.dram_tensor("GD", [D, O_LOC], BF16, kind="ExternalInput").ap()
    VD = nc.dram_tensor("VD", [D, O_LOC], F32, kind="ExternalInput").ap()
    G0D = nc.dram_tensor("G0D", [1, O_LOC], BF16, kind="ExternalInput").ap()
    ONE1 = nc.dram_tensor("ONE1", [1, 128], BF16, kind="ExternalInput").ap()
    OUT = nc.dram_tensor("OUT", [O_LOC, B, L, L], BF16, kind="ExternalOutput").ap()

    with tile.TileContext(nc) as tc:
        with (
            tc.tile_pool(name="const", bufs=1) as cst,
            tc.tile_pool(name="w", bufs=3) as wpool,
            tc.tile_pool(name="ut", bufs=3) as utpool,
            tc.tile_pool(name="cs", bufs=2) as cspool,
        ):
            # ---- resident inputs (order = DMA queue order; X1/G/W0 first so
            # the D1X precompute and A(0) can start as early as possible) -----
            Ws_tiles = {}
            UT_tiles = {}

            def dma_w(op):
                # one DMA per og-PAIR: [i%128, oi, it, j]
                ws = wpool.tile([128, 2, 4, D], BF16, tag="ws", name=f"ws{op}")
                nc.sync.dma_start(
                    out=ws[:],
                    in_=WMD[2 * op:2 * op + 2].rearrange(
                        "oi (it p) j -> p oi it j", p=128),
                )
                Ws_tiles[op] = ws

            # startup: X1 + small consts on the Act DGE queue, W on the SP
            # queue — the two run concurrently, so A(0) starts ~5us in.
            X1s = cst.tile([128, 4, BL], BF16, tag="x1s")     # [i%128, it, b*256+x]
            nc.scalar.dma_start(out=X1s[:], in_=X1T.rearrange("(it p) c -> p it c", p=128))
            ws0 = wpool.tile([128, 2, 4, D], BF16, tag="ws", name="ws0")
            nc.sync.dma_start(out=ws0[:, 0], in_=WMD[0].rearrange("(it p) j -> p it j", p=128))
            nc.sync.dma_start(out=ws0[:, 1], in_=WMD[1].rearrange("(it p) j -> p it j", p=128))
            Ws_tiles[0] = ws0
            Gs = cst.tile([128, 4, O_LOC], BF16, tag="gs")
            nc.scalar.dma_start(out=Gs[:], in_=GD.rearrange("(it p) o -> p it o", p=128))
            g0row = cst.tile([1, O_LOC], BF16, tag="g0row")
            nc.scalar.dma_start(out=g0row[:], in_=G0D[:])
            one1 = cst.tile([1, 128], BF16, tag="one1")
            nc.scalar.dma_start(out=one1[:], in_=ONE1[:])
            Vs = cst.tile([128, 4, O_LOC], F32, tag="vs")
            nc.scalar.dma_start(out=Vs[:], in_=VD.rearrange("(jt p) o -> p jt o", p=128))
            dma_w(1)
            X2s = cst.tile([128, 4, BL], BF16, tag="x2s")     # [j%128, jt, b*256+y]
            nc.sync.dma_start(out=X2s[:], in_=X2T.rearrange("(jt p) c -> p jt c", p=128))

            # ---- main loop: A(og) interleaved with B(og-1) ------------------
            psA_ctx = tc.tile_pool(name="psA", bufs=4, space="PSUM")
            psB_ctx = tc.tile_pool(name="psB", bufs=3, space="PSUM")
            psD_ctx = tc.tile_pool(name="psD", bufs=1, space="PSUM")
            psA = psA_ctx.__enter__()
            psB = psB_ctx.__enter__()
            psD = psD_ctx.__enter__()

            D1X = cst.tile([128, B, 2, O_LOC], F32, tag="d1x")

            def d1x_group(k):
                # D1X[b][x-part, xt, o] incl. +g0 (via rank-1 K=1 matmul);
                # one tiny group, interleaved between A(0) groups so the
                # single psD buffer always has eviction slack.
                b, xt = divmod(k, 2)
                pd = psD.tile([128, O_LOC], F32, tag="pd")
                c0 = b * L + xt * 128
                for it in range(4):
                    nc.tensor.matmul(
                        pd[:],
                        lhsT=X1s[:, it, c0:c0 + 128],
                        rhs=Gs[:, it, :],
                        start=(it == 0), stop=False,
                    )
                nc.tensor.matmul(
                    pd[:],
                    lhsT=one1[0:1, :],
                    rhs=g0row[0:1, :],
                    start=False, stop=True,
                )
                nc.scalar.copy(out=D1X[:, b, xt, :], in_=pd[:])

            def step_a(og):
                # UT'[j-part, jt, c] = sum_i W[i,j] x1[i,c]  (+V on evict)
                ut = utpool.tile([128, 4, BL], BF16, tag="ut", name=f"ut{og}")
                ws = Ws_tiles[og // 2]
                oi = og % 2
                ev = 0
                for jt in range(4):
                    for bp in range(2):
                        pa = psA.tile([128, 512], F32, tag="pa")
                        for it in range(4):
                            nc.tensor.matmul(
                                pa[:],
                                lhsT=ws[:, oi, it, jt * 128:(jt + 1) * 128],
                                rhs=X1s[:, it, bp * 512:(bp + 1) * 512],
                                start=(it == 0), stop=(it == 3),
                            )
                        # evict + V[j, og]; alternate DVE/Act to balance
                        dst = ut[:, jt, bp * 512:(bp + 1) * 512]
                        if ev % 2 == 0:
                            nc.vector.tensor_scalar_add(
                                dst, pa[:], Vs[:, jt, og:og + 1])
                        else:
                            nc.scalar.add(dst, pa[:], Vs[:, jt, og:og + 1])
                        if og == 0:
                            d1x_group(ev)
                        ev += 1
                UT_tiles[og] = ut

            def step_b(og):
                ut = UT_tiles.pop(og)
                cs = cspool.tile([128, B, 2, 256], BF16, tag="cs")
                ev = 0
                for b in range(B):
                    pb = psB.tile([128, 512], F32, tag="pb")
                    for xt in range(2):
                        for jt in range(4):
                            nc.tensor.matmul(
                                pb[:, xt * 256:(xt + 1) * 256],
                                lhsT=ut[:, jt, b * L + xt * 128: b * L + xt * 128 + 128],
                                rhs=X2s[:, jt, b * L:(b + 1) * L],
                                start=(jt == 0), stop=(jt == 3),
                            )
                    for xt in range(2):
                        src = pb[:, xt * 256:(xt + 1) * 256]
                        bias = D1X[:, b, xt, og:og + 1]
                        if ev % 2 == 0:
                            nc.scalar.add(cs[:, b, xt, :], src, bias)
                        else:
                            nc.vector.tensor_scalar_add(cs[:, b, xt, :], src, bias)
                        ev += 1
                # one batched DMA for the whole og slab: [b, x, y]
                nc.sync.dma_start(
                    out=OUT[og].rearrange("b (xt p) y -> p b xt y", p=128),
                    in_=cs[:],
                )

            step_a(0)
            for og in range(1, O_LOC):
                if og % 2 == 1 and og // 2 + 2 < O_LOC // 2:
                    dma_w(og // 2 + 2)
                step_a(og)
                step_b(og - 1)
            step_b(O_LOC - 1)

            psD_ctx.__exit__(None, None, None)
            psB_ctx.__exit__(None, None, None)
            psA_ctx.__exit__(None, None, None)

    if split_waits:
        _split_multi_waits(nc)
    return nc


_NC_CACHE = None


def _get_nc():
    global _NC_CACHE
    if _NC_CACHE is None:
        _NC_CACHE = build_nc()
    return _NC_CACHE


def _prep_inputs(x1, x2, bw, W_bil, W_lin, b_lin):
    """Host-side glue: softmax of the 2-vector, per-core slicing/layout."""
    x1 = np.asarray(x1, np.float32)
    x2 = np.asarray(x2, np.float32)
    bw = np.asarray(bw, np.float64)
    W_bil = np.asarray(W_bil, np.float32)
    W_lin = np.asarray(W_lin, np.float32)
    b_lin = np.asarray(b_lin, np.float32)

    e = np.exp(bw - bw.max())
    bwn = (e / e.sum()).astype(np.float32)
    bwn0, bwn1 = float(bwn[0]), float(bwn[1])

    bf = ml_dtypes.bfloat16
    x1T = np.ascontiguousarray(x1.transpose(2, 0, 1).reshape(D, BL)).astype(bf)
    x2T = np.ascontiguousarray(x2.transpose(2, 0, 1).reshape(D, BL)).astype(bf)
    one1 = np.ones((1, 128), bf)

    in_maps = []
    for c in range(N_CORES):
        o_sl = slice(c * O_LOC, (c + 1) * O_LOC)
        Wb = W_bil[o_sl]                                   # [16, 513, 513]
        WMD = np.ascontiguousarray(bwn0 * Wb[:, :D, :D]).astype(bf)
        G = (bwn0 * Wb[:, :D, D].T + bwn1 * W_lin[:D, o_sl]).astype(bf)
        V = (bwn0 * Wb[:, D, :D].T + bwn1 * W_lin[D:, o_sl]).astype(np.float32)
        G0 = (bwn0 * Wb[:, D, D] + bwn1 * b_lin[o_sl]).reshape(1, O_LOC).astype(bf)
        in_maps.append({
            "WMD": WMD, "X1T": x1T, "X2T": x2T,
            "GD": np.ascontiguousarray(G),
            "VD": np.ascontiguousarray(V),
            "G0D": np.ascontiguousarray(G0), "ONE1": one1,
        })
    return in_maps


def _assemble(results):
    out = np.empty((B, L, L, O), np.float32)
    for c in range(N_CORES):
        # per-core OUT is [o_local, b, x, y] -> full is [b, x, y, o]
        out[:, :, :, c * O_LOC:(c + 1) * O_LOC] = \
            np.asarray(results[c]["OUT"]).astype(np.float32).transpose(1, 2, 3, 0)
    return out


def kernel(**inputs):
    in_maps = _prep_inputs(**inputs)
    nc = _get_nc()
    res = run_bass_kernel_spmd(nc, in_maps, list(range(N_CORES)))
    return _assemble(res.results)
